# revision 1
# baseline (speedup 1.0000x reference)
"""Trainium2 Bass kernel for the masked MQA attention block (nn_Attention_4252017623134).

Sharding: pure data-parallel over batch. b=8 batch elements, 8 NeuronCores,
one batch element per core, weights replicated. No collectives.

Per-core math (n=1024, d=1024, h=16, dh=64, inner=1024):
  context = x                      (pre-norm residual branch feeds K/V)
  xn  = layernorm(x) * g_in
  q   = xn @ Wq.T   (per head, scaled by 1/8 = dh^-0.5, folded into exp scale)
  k,v = context @ Wkv.T (single shared KV head) + null_kv token appended
  att = softmax(mask(q k^T / 8))   (padding + causal(key j' visible iff j' <= i) masks)
  out = layernorm(att @ v @ Wo.T) * g_out

Key design decisions:
  * All matmul operands live in the transposed (feature-major) domain so every
    contraction has its contraction dim on partitions. Host supplies
    x^T / Wq^T / Wkv^T / Wo^T (pure layout marshaling).
  * Scores are computed transposed: simT[j, i] = k_j . q_i on the PE
    (lhsT = kT tile, rhs = qT), evacuated from PSUM through the scalar
    engine as exp(0.125 * s + pad_bias) directly — the softmax max-shift is
    mathematically unnecessary here (logits are ~N(0,1); exp cannot overflow)
    and padding masking is a free per-partition bias. Causal masking is a
    cheap 0/1 triangular multiply on the single partially-masked 128-column
    diagonal band; fully-masked (t, i) blocks are never computed at all.
  * P @ V runs transposed too: outT[c, i] accumulated over j-tiles with the
    natural v layout as the stationary operand, with an appended ones column
    so the softmax denominator drops out of the same matmuls for free.
  * The denominator division happens on the small [inner, n] attention
    output (1M elements, not the 17M-element probability matrix), using a
    DRAM round-trip broadcast of the reciprocal rows.
  * Matmul operands are float32r (TF32-class, 1 row/cycle at N>=256,
    ~16x more accurate than bf16); LN/softmax/statistics stay fp32.
"""

import contextlib

import numpy as np

import concourse.bass as bass
import concourse.bacc as bacc
import concourse.tile as tile
import concourse.mybir as mybir
from concourse.bass_utils import run_bass_kernel_spmd
from concourse.masks import make_identity

N = 1024          # sequence length per core
D = 1024          # model dim
H = 16            # query heads
DH = 64           # head dim
INNER = H * DH    # 1024
NT = N // 128     # 8 i-tiles / j-tiles / d-tiles
EPS = 1e-5
MASK_BIAS = -30000.0  # additive pad-mask bias; exp(-30000) == 0.0 in fp32

F32 = mybir.dt.float32
F32R = mybir.dt.float32r
U8 = mybir.dt.uint8
AF = mybir.ActivationFunctionType
ALU = mybir.AluOpType


def _emit(nc):
    # ---------------- DRAM I/O ----------------
    x_d = nc.dram_tensor("x", [N, D], F32, kind="ExternalInput")
    xT_d = nc.dram_tensor("xT", [D, N], F32, kind="ExternalInput")
    wqT_d = nc.dram_tensor("wqT", [D, INNER], F32, kind="ExternalInput")
    wkvT_d = nc.dram_tensor("wkvT", [D, 2 * DH], F32, kind="ExternalInput")
    woT_d = nc.dram_tensor("woT", [INNER, D], F32, kind="ExternalInput")
    nkv_d = nc.dram_tensor("nkv", [2, DH], F32, kind="ExternalInput")
    mask_d = nc.dram_tensor("mask", [N], U8, kind="ExternalInput")
    gin_d = nc.dram_tensor("gin", [D], F32, kind="ExternalInput")
    gout_d = nc.dram_tensor("gout", [D], F32, kind="ExternalInput")
    out_d = nc.dram_tensor("out", [N, D], F32, kind="ExternalOutput")
    ddram = nc.dram_tensor("dscratch", [H, N], F32)        # denominator round-trip
    unull_dram = nc.dram_tensor("unull_scratch", [H, N], F32R)  # null-row per-head fetch

    d_ = dict(x_d=x_d, xT_d=xT_d, wqT_d=wqT_d, wkvT_d=wkvT_d, woT_d=woT_d,
              nkv_d=nkv_d, mask_d=mask_d, gin_d=gin_d, gout_d=gout_d,
              out_d=out_d, ddram=ddram, unull_dram=unull_dram)
    with tile.TileContext(nc) as tc:
        _emit_tile(nc, tc, d_)
    return nc


def _emit_tile(nc, tc, d_):
    x_d, xT_d = d_["x_d"], d_["xT_d"]
    wqT_d, wkvT_d, woT_d = d_["wqT_d"], d_["wkvT_d"], d_["woT_d"]
    nkv_d, mask_d = d_["nkv_d"], d_["mask_d"]
    gin_d, gout_d, out_d = d_["gin_d"], d_["gout_d"], d_["out_d"]
    ddram, unull_dram = d_["ddram"], d_["unull_dram"]

    ctx = contextlib.ExitStack()
    with ctx:
        consts = ctx.enter_context(tc.tile_pool(name="consts", bufs=1))
        persist = ctx.enter_context(tc.tile_pool(name="persist", bufs=1))
        stage = ctx.enter_context(tc.tile_pool(name="stage", bufs=4))

        # ---------------- constants ----------------
        ident = consts.tile([128, 128], F32)
        make_identity(nc, ident[:])
        # causal 0/1 band mask (fp32; multiplies f32r u tiles): keep iff f >= p
        mtri = consts.tile([128, 128], F32)
        nc.gpsimd.memset(mtri[:], 1.0)
        nc.gpsimd.affine_select(out=mtri[:], in_=mtri[:], compare_op=ALU.is_ge,
                                fill=0.0, base=0, pattern=[[1, 128]],
                                channel_multiplier=-1)
        # padding bias per key tile: mask==1 -> 0, mask==0 -> MASK_BIAS
        mask_u8 = consts.tile([128, NT], U8)
        nc.sync.dma_start(out=mask_u8[:],
                          in_=bass.AP(tensor=mask_d, offset=0,
                                      ap=[[1, 128], [128, NT]]))
        pbias = consts.tile([128, NT], F32)
        nc.vector.tensor_scalar(out=pbias[:], in0=mask_u8[:], scalar1=1,
                                scalar2=-MASK_BIAS, op0=ALU.subtract,
                                op1=ALU.mult)
        # g_in per d-tile (fold into WqT), g_out broadcast tile (final LN)
        gin_t = consts.tile([128, NT], F32)
        nc.sync.dma_start(out=gin_t[:],
                          in_=bass.AP(tensor=gin_d, offset=0,
                                      ap=[[1, 128], [128, NT]]))
        eps_t = consts.tile([128, 1], F32)
        nc.vector.memset(eps_t[:], EPS)
        # warm the ACT function tables (Sqrt/Exp) so the first LN/softmax
        # doesn't pay the ~1.3us table load inside its dependency chain
        warm = consts.tile([128, 2], F32)
        nc.scalar.activation(out=warm[:, 0:1], in_=eps_t[:], func=AF.Sqrt)
        nc.scalar.activation(out=warm[:, 1:2], in_=eps_t[:], func=AF.Exp)
        ones_t = consts.tile([128, 2], F32)
        nc.vector.memset(ones_t[:], 1.0)
        zeros_t = consts.tile([128, 2], F32)
        nc.vector.memset(zeros_t[:], 0.0)
        zeros128 = consts.tile([128, 128], F32)
        nc.vector.memset(zeros128[:], 0.0)

        # ------------- whole-kernel persistent tiles -------------
        kT2 = persist.tile([128, N + 1], F32R, tag="kT2")   # both 64-halves = k^T (+null col)
        v_aug = persist.tile([128, NT, DH + 1], F32R, tag="v_aug")  # [128, t, 65], col 64 = 1
        vnull_aug = persist.tile([1, DH + 1], F32R, tag="vnull")    # [v_null | 1] at partition 0
        outT = persist.tile([128, NT, N], F32R, tag="outT")  # attn out^T (chunk m = heads 2m,2m+1)

        # ============ Phases A+B window (qT lives across both) ============
        with tc.tile_pool(name="poolAB", bufs=1) as poolAB:
            qT = poolAB.tile([128, NT, N], F32R, tag="qT")

            # ---- Phase A: LN1, transposes, projections ----
            with tc.tile_pool(name="poolA", bufs=1) as poolA, \
                 tc.tile_pool(name="xtr", bufs=2) as xtr_pool, \
                 tc.tile_pool(name="psA2", bufs=2, space="PSUM") as psA2, \
                 tc.tile_pool(name="psAn", bufs=1, space="PSUM") as psAn:
                with tc.tile_pool(name="psA", bufs=3, space="PSUM") as psA:
                  xnT = poolA.tile([128, NT, N], F32R, tag="xnT")
                  wqT = poolA.tile([128, NT, INNER], F32R, tag="wqT")
                  wkvT = poolA.tile([128, NT, 2 * DH], F32R, tag="wkvT")
                  vT_s = poolA.tile([64, N], F32, tag="vT_s")

                  # --- x: LN stats + xn, transpose to xnT ---
                  for it in range(NT):
                      x_s = stage.tile([128, D], F32, tag="ldx")
                      nc.sync.dma_start(out=x_s[:], in_=x_d[it * 128:(it + 1) * 128, :])
                      st = stage.tile([128, 2, 6], F32, tag="bnst")
                      nc.vector.bn_stats(out=st[:, 0, :], in_=x_s[:, 0:512])
                      nc.vector.bn_stats(out=st[:, 1, :], in_=x_s[:, 512:1024])
                      mv = stage.tile([128, 2], F32, tag="bnmv")
                      nc.vector.bn_aggr(out=mv[:], in_=st[:])
                      rstd = stage.tile([128, 1], F32, tag="rstd")
                      nc.scalar.activation(out=rstd[:], in_=mv[:, 1:2], func=AF.Sqrt,
                                           bias=eps_t[:], scale=1.0)
                      nc.vector.reciprocal(out=rstd[:], in_=rstd[:])
                      xn_s = stage.tile([128, D], F32, tag="xnft")
                      nc.vector.tensor_scalar(out=xn_s[:], in0=x_s[:],
                                              scalar1=mv[:, 0:1], scalar2=rstd[:],
                                              op0=ALU.subtract, op1=ALU.mult)
                      for g in range(2):
                          pt = psA.tile([128, 4, 128], F32, tag="tp")
                          for q_ in range(4):
                              dt_ = 4 * g + q_
                              nc.tensor.transpose(pt[:, q_, :],
                                                  xn_s[:, dt_ * 128:(dt_ + 1) * 128],
                                                  ident[:])
                          nc.scalar.copy(
                              xnT[:, 4 * g:4 * g + 4, it * 128:(it + 1) * 128], pt[:])

                  # --- load + round kv weights ---
                  wkv_s = stage.tile([128, NT, 2 * DH], F32, tag="ld")
                  nc.sync.dma_start(out=wkv_s[:],
                                    in_=wkvT_d.ap().rearrange("(t p) c -> p t c", p=128))
                  for t in range(NT):
                      nc.vector.tensor_copy(wkvT[:, t, :], wkv_s[:, t, :])

                  # --- kv projection: kvT[c, j] accumulated over d-tiles ---
                  pkv0 = psA2.tile([128, 512], F32, tag="mm512")
                  pkv1 = psA2.tile([128, 512], F32, tag="mm512")
                  pkv = [pkv0, pkv1]
                  for t in range(NT):
                      xt_s = stage.tile([128, N], F32, tag="ldx")
                      nc.sync.dma_start(out=xt_s[:], in_=xT_d[t * 128:(t + 1) * 128, :])
                      xr = xtr_pool.tile([128, N], F32R, tag="xT_r")
                      nc.vector.tensor_copy(xr[:], xt_s[:])
                      for ch in range(2):
                          nc.tensor.matmul(pkv[ch][:], wkvT[:, t, :],
                                           xr[:, ch * 512:(ch + 1) * 512],
                                           start=(t == 0), stop=(t == NT - 1))
                  for ch in range(2):
                      cs = slice(ch * 512, (ch + 1) * 512)
                      nc.scalar.copy(kT2[0:64, cs], pkv[ch][0:64, :])
                      nc.scalar.copy(kT2[64:128, cs], pkv[ch][0:64, :])
                      nc.vector.tensor_copy(vT_s[:, cs], pkv[ch][64:128, :])
                  # null k column (both halves)
                  nk_s = stage.tile([64, 2], F32, tag="nk")
                  nc.sync.dma_start(out=nk_s[:],
                                    in_=nkv_d.ap().rearrange("a c -> c a"))
                  nc.scalar.copy(kT2[0:64, N:N + 1], nk_s[:, 0:1])
                  nc.scalar.copy(kT2[64:128, N:N + 1], nk_s[:, 0:1])
                  # v: transpose vT -> natural [j, c] tiles, append ones column
                  for t in range(NT):
                      pv_t = psA.tile([128, 128], F32, tag="tp")
                      nc.tensor.transpose(pv_t[0:128, 0:64],
                                          vT_s[:, t * 128:(t + 1) * 128],
                                          ident[0:64, 0:64])
                      nc.scalar.copy(v_aug[:, t, 0:DH], pv_t[0:128, 0:64])
                      nc.vector.tensor_copy(v_aug[:, t, DH:DH + 1], ones_t[:, 0:1])
                  # vnull_aug = [v_null | 1] at partition 0 (DMA row straight from DRAM)
                  vn_s = stage.tile([1, DH], F32, tag="nk2")
                  nc.sync.dma_start(out=vn_s[:], in_=nkv_d[1:2, :])
                  nc.vector.tensor_copy(vnull_aug[0:1, 0:DH], vn_s[:])
                  nc.vector.tensor_copy(vnull_aug[0:1, DH:DH + 1], ones_t[0:1, 0:1])

                # --- load + round q weights ---
                for t in range(NT):
                    w_s = stage.tile([128, INNER], F32, tag="ld")
                    nc.sync.dma_start(out=w_s[:], in_=wqT_d[t * 128:(t + 1) * 128, :])
                    # round to f32r with g_in folded in (per-partition scalar)
                    nc.vector.tensor_scalar_mul(wqT[:, t, :], w_s[:], gin_t[:, t:t + 1])

                # --- q projection: qT = (Wq*g_in) @ xn^T ---
                for m in range(NT):
                    ms = slice(m * 128, (m + 1) * 128)
                    for ch in range(2):
                        pq = psA2.tile([128, 512], F32, tag="mm512")
                        for t in range(NT):
                            nc.tensor.matmul(pq[:], wqT[:, t, ms],
                                             xnT[:, t, ch * 512:(ch + 1) * 512],
                                             start=(t == 0), stop=(t == NT - 1))
                        nc.vector.tensor_copy(qT[:, m, ch * 512:(ch + 1) * 512], pq[:])

                # --- null-token scores for all heads, one [16, N] tile ---
                # per-pair lhsT with only cols 2m/2m+1 nonzero; accumulating
                # MMs build all 16 rows so ONE exp evacuates everything
                knulls = []
                for m in range(NT):
                    kn = poolA.tile([128, H], F32R, tag=f"knull16_{m}")
                    nc.vector.tensor_copy(kn[:], zeros128[:, 0:H])
                    nc.scalar.copy(kn[0:64, 2 * m:2 * m + 1], nk_s[:, 0:1])
                    nc.scalar.copy(kn[64:128, 2 * m + 1:2 * m + 2], nk_s[:, 0:1])
                    knulls.append(kn)
                pnull = psAn.tile([H, N], F32, tag="mmnull")
                for ch in range(2):
                    for m in range(NT):
                        nc.tensor.matmul(pnull[:, ch * 512:(ch + 1) * 512],
                                         knulls[m][:], qT[:, m, ch * 512:(ch + 1) * 512],
                                         start=(m == 0), stop=(m == NT - 1))
                uall = stage.tile([H, N], F32R, tag="ld")
                nc.scalar.activation(uall[:], pnull[:], AF.Exp, scale=0.125)
                nc.sync.dma_start(out=unull_dram.ap(), in_=uall[:])

            # ---- Phase B: attention (scores + exp + PV) ----
            # visibility: key j' visible to query i iff i >= j'
            s_t = [128 * t for t in range(NT)]
            with tc.tile_pool(name="psB", bufs=3, space="PSUM") as psB, \
                 tc.tile_pool(name="psPV", bufs=2, space="PSUM") as psPV, \
                 tc.tile_pool(name="upool", bufs=12) as upool, \
                 tc.tile_pool(name="unh", bufs=3) as unh_pool:
                for m in range(NT):          # head pairs
                    for ph in range(2):      # parity within pair
                        h = 2 * m + ph
                        base = 64 * ph
                        unull_h = unh_pool.tile([1, N], F32R, tag="unh")
                        nc.sync.dma_start(out=unull_h[:],
                                          in_=unull_dram[h:h + 1, :])
                        dd_h = unh_pool.tile([1, N], F32, tag="ddh")
                        utiles = []
                        for t in range(NT):
                            ps = psB.tile([128, N], F32, tag="scores")
                            lo = s_t[t]
                            if lo < 512:
                                # widen short tails to 256 cols: f32r runs 4x
                                # slower below 256; extra cols land outside the
                                # exp window and are never read
                                mlo = min(lo, 256)
                                nc.tensor.matmul(ps[:, mlo:512],
                                                 kT2[base:base + 64, t * 128:(t + 1) * 128],
                                                 qT[base:base + 64, m, mlo:512],
                                                 start=True, stop=True)
                            # narrow the high chunk to the visible range
                            # (>=256 wide for f32r; t=7 extends to 768 and the
                            #  [768,896) strip is zeroed below for the PV ext)
                            elo = min(max(512, lo), 768)
                            nc.tensor.matmul(ps[:, elo:1024],
                                             kT2[base:base + 64, t * 128:(t + 1) * 128],
                                             qT[base:base + 64, m, elo:1024],
                                             start=True, stop=True)
                            u = upool.tile([128, N], F32R, tag="u")
                            nc.scalar.activation(u[:, lo:N], ps[:, lo:N], AF.Exp,
                                                 bias=pbias[:, t:t + 1], scale=0.125)
                            # causal diagonal band
                            nc.vector.tensor_mul(u[:, lo:lo + 128],
                                                 u[:, lo:lo + 128], mtri[:])
                            if lo in (384, 896):
                                # zero strip so the PV matmul can start 128 cols
                                # early (N=256 keeps f32r at full rate)
                                nc.vector.tensor_copy(u[:, lo - 128:lo], zeros128[:])
                            utiles.append(u)
                        # PV accumulation, one 1-bank tile per i-chunk
                        for ch in range(2):
                            clo, chi_ = ch * 512, (ch + 1) * 512
                            pv = psPV.tile([65, 512], F32, tag="pv")
                            first = True
                            for t in range(NT):
                                lo = max(s_t[t], clo)
                                if lo >= chi_:
                                    continue
                                if chi_ - lo == 128:
                                    lo -= 128  # strip was zeroed above
                                nc.tensor.matmul(pv[:, lo - clo:512],
                                                 v_aug[:, t, :],
                                                 utiles[t][:, lo:chi_],
                                                 start=first, stop=False)
                                first = False
                            # null token contribution (K=1)
                            nc.tensor.matmul(pv[:], vnull_aug[0:1, :],
                                             unull_h[0:1, clo:chi_],
                                             start=False, stop=True)
                            cs = slice(clo, chi_)
                            nc.vector.tensor_copy(outT[base:base + 64, m, cs], pv[0:64, :])
                            nc.vector.tensor_copy(dd_h[0:1, cs], pv[64:65, :])
                        nc.sync.dma_start(out=ddram[h:h + 1, :], in_=dd_h[:])
                    # divide this pair's outT rows by the softmax denominators
                    dv = unh_pool.tile([128, N], F32, tag="dv")
                    nc.sync.dma_start(out=dv[0:64, :],
                                      in_=bass.AP(tensor=ddram, offset=(2 * m) * N,
                                                  ap=[[0, 64], [1, N]]))
                    nc.sync.dma_start(out=dv[64:128, :],
                                      in_=bass.AP(tensor=ddram, offset=(2 * m + 1) * N,
                                                  ap=[[0, 64], [1, N]]))
                    nc.vector.reciprocal(out=dv[:], in_=dv[:])
                    nc.gpsimd.tensor_mul(outT[:, m, :], outT[:, m, :], dv[:])

        # ============ Phase C: out-projection (natural layout) + LN2 ====
        with tc.tile_pool(name="psC", bufs=4, space="PSUM") as psC, \
             tc.tile_pool(name="poolC", bufs=1) as poolC, \
             tc.tile_pool(name="poolC2", bufs=2) as poolC2:
            woT = poolC.tile([128, NT, D], F32R, tag="woT")
            for t in range(NT):
                wo_s = stage.tile([128, D], F32, tag="ld")
                nc.sync.dma_start(out=wo_s[:], in_=woT_d[t * 128:(t + 1) * 128, :])
                nc.vector.tensor_copy(woT[:, t, :], wo_s[:])
            gout_b = poolC.tile([128, D], F32, tag="gout_b")
            nc.sync.dma_start(out=gout_b[:],
                              in_=bass.AP(tensor=gout_d, offset=0,
                                          ap=[[0, 128], [1, D]]))
            # out-projection with per-i-tile LN2 (rotating buffer);
            # bn_stats per chunk so stats overlap the other chunk's matmuls
            for it in range(NT):
                i_s = slice(it * 128, (it + 1) * 128)
                fx = poolC2.tile([128, D], F32, tag="fn")
                st = stage.tile([128, 2, 6], F32, tag="bnst")
                for ch in range(2):
                    po = psC.tile([128, 512], F32, tag="mm512c")
                    for ct in range(NT):
                        nc.tensor.matmul(po[:], outT[:, ct, i_s],
                                         woT[:, ct, ch * 512:(ch + 1) * 512],
                                         start=(ct == 0), stop=(ct == NT - 1))
                    nc.scalar.copy(fx[:, ch * 512:(ch + 1) * 512], po[:])
                    nc.vector.bn_stats(out=st[:, ch, :],
                                       in_=fx[:, ch * 512:(ch + 1) * 512])
                mv = stage.tile([128, 2], F32, tag="bnmv")
                nc.vector.bn_aggr(out=mv[:], in_=st[:])
                rstd = stage.tile([128, 1], F32, tag="rstd")
                nc.scalar.activation(out=rstd[:], in_=mv[:, 1:2], func=AF.Sqrt,
                                     bias=eps_t[:], scale=1.0)
                nc.vector.reciprocal(out=rstd[:], in_=rstd[:])
                o_s = stage.tile([128, D], F32, tag="xnft")
                geng = nc.vector if it >= NT - 2 else nc.gpsimd
                for ch2 in range(2):
                    cs2 = slice(ch2 * 512, (ch2 + 1) * 512)
                    nc.vector.tensor_scalar(out=o_s[:, cs2], in0=fx[:, cs2],
                                            scalar1=mv[:, 0:1], scalar2=rstd[:],
                                            op0=ALU.subtract, op1=ALU.mult)
                    geng.tensor_mul(o_s[:, cs2], o_s[:, cs2], gout_b[:, cs2])
                    nc.sync.dma_start(out=out_d[it * 128:(it + 1) * 128, cs2],
                                      in_=o_s[:, cs2])


_CACHED = None


def _get_nc():
    global _CACHED
    if _CACHED is None:
        nc = bacc.Bacc("TRN2", target_bir_lowering=False, debug=False)
        _emit(nc)
        nc.compile()
        _CACHED = nc
    return _CACHED


def make_in_maps(x, mask, g_in, Wq, Wkv, null_kv, Wo, g_out):
    b = x.shape[0]
    xT = np.ascontiguousarray(np.transpose(x, (0, 2, 1)))
    mask_u8 = np.ascontiguousarray(mask).view(np.uint8) if mask.dtype == np.bool_ \
        else mask.astype(np.uint8)
    shared = {
        "wqT": np.ascontiguousarray(Wq.T),
        "wkvT": np.ascontiguousarray(Wkv.T),
        "woT": np.ascontiguousarray(Wo.T),
        "nkv": np.ascontiguousarray(null_kv.astype(np.float32)),
        "gin": np.ascontiguousarray(g_in.astype(np.float32)),
        "gout": np.ascontiguousarray(g_out.astype(np.float32)),
    }
    return [
        {"x": np.ascontiguousarray(x[c]), "xT": xT[c], "mask": mask_u8[c], **shared}
        for c in range(b)
    ]


def kernel(x, mask, g_in, Wq, Wkv, null_kv, Wo, g_out):
    x = np.asarray(x)
    mask = np.asarray(mask)
    g_in, g_out = np.asarray(g_in), np.asarray(g_out)
    Wq, Wkv, Wo = np.asarray(Wq), np.asarray(Wkv), np.asarray(Wo)
    null_kv = np.asarray(null_kv)
    b = x.shape[0]
    assert x.shape == (b, N, D) and b == 8
    in_maps = make_in_maps(x, mask, g_in, Wq, Wkv, null_kv, Wo, g_out)
    nc = _get_nc()
    res = run_bass_kernel_spmd(nc, in_maps, core_ids=list(range(b)))
    return np.stack([res.results[c]["out"] for c in range(b)], axis=0)



# revision 14
# speedup vs baseline: 1.1048x; 1.1048x over previous
"""Trainium2 Bass kernel for the masked MQA attention block (nn_Attention_4252017623134).

Sharding: pure data-parallel over batch. b=8 batch elements, 8 NeuronCores,
one batch element per core, weights replicated. No collectives.

Per-core math (n=1024, d=1024, h=16, dh=64, inner=1024):
  context = x                      (pre-norm residual branch feeds K/V)
  xn  = layernorm(x) * g_in
  q   = xn @ Wq.T   (per head, scaled by 1/8 = dh^-0.5, folded into exp scale)
  k,v = context @ Wkv.T (single shared KV head) + null_kv token
  att = softmax(mask(q k^T / 8))   (padding + causal(key j visible iff j <= i))
  out = layernorm(att @ v @ Wo.T) * g_out

Key design decisions (v2):
  * All matmuls in bf16 (1 PE row/cycle at ANY width, vs f32r needing >=256).
  * LN1 is folded into the q-projection: q_i = r_i * (Wq'' @ x_i) with
    Wq'' = Wq*diag(g) - outer(Wq@g, 1)/D precomputed on HOST (mean term) and
    r_i = rsqrt(var_i + eps) applied as a per-column scale at PSUM evacuation.
    This removes all 64 xn transposes and the LN->transpose->matmul serial
    dependency; q projects straight from the host-marshaled x^T.
  * Null-token scores become 16 extra projection output channels
    (wn[d,h] = sum_dh nk_dh Wq''[h*64+dh, d], host-computed): no separate
    null machinery, no DRAM round trips.
  * Padding mask is applied by ZEROING masked key/value columns instead of
    an exp bias: masked j gives logit 0 -> u=1, contributes v_j=0 to the
    numerator and is excluded from the denominator via a mask column
    appended to V. Exp therefore needs NO per-partition bias, so one
    activation call can cover score tiles of multiple j-tiles.
  * Scores are computed transposed (simT[j,i], exact visible windows only);
    P@V runs in NATURAL layout: lhsT = u-tile [j, i-chunk], rhs = v [j, 65]
    (64 channels + mask column). Cost per accumulation step is 65 cycles
    instead of a full i-window: PV drops from ~37us to ~19us. The softmax
    denominator lands in PSUM column 64; division is fused into the PV
    evacuation as a per-partition reciprocal multiply.
  * PV output [i, c] is transposed back to [c, i] via 64 bf16 PE transposes
    (128x128 head-pairs) for the out-projection, which needs c on partitions.
  * LN2 runs on the out-projection PSUM directly; g_out applied on gpsimd.
"""

import contextlib

import numpy as np
import ml_dtypes

import concourse.bass as bass
import concourse.bacc as bacc
import concourse.tile as tile
import concourse.mybir as mybir
from concourse.bass_utils import run_bass_kernel_spmd
from concourse.masks import make_identity

N = 1024          # sequence length per core
D = 1024          # model dim
H = 16            # query heads
DH = 64           # head dim
INNER = H * DH    # 1024
NT = N // 128     # 8 i-tiles / j-tiles / d-tiles
EPS = 1e-5

F32 = mybir.dt.float32
BF16 = mybir.dt.bfloat16
U8 = mybir.dt.uint8
AF = mybir.ActivationFunctionType
ALU = mybir.AluOpType

# exp groups per head: (j-tiles, total cols). Windows are [128*t, N) so widths
# are 1024,896,768,640,512,384,256,128; paired to 1024-col (2 psum banks) tiles.
EXP_GROUPS = [(0,), (1, 7), (2, 6), (3, 5), (4,)]
# u-tile column offset of each j-tile's window inside the per-head u buffer
U_OFF = {}
_off = 0
for _g in EXP_GROUPS:
    for _t in _g:
        U_OFF[_t] = _off
        _off += N - 128 * _t
U_COLS = _off  # 4608


def _emit(nc):
    # ---------------- DRAM I/O ----------------
    xT_d = nc.dram_tensor("xT", [D, N], BF16, kind="ExternalInput")
    x_d = nc.dram_tensor("x", [N, D], BF16, kind="ExternalInput")
    wqT_d = nc.dram_tensor("wqT", [D, INNER], BF16, kind="ExternalInput")
    wnT_d = nc.dram_tensor("wnT", [D, H], BF16, kind="ExternalInput")
    wkvT_d = nc.dram_tensor("wkvT", [D, 2 * DH], BF16, kind="ExternalInput")
    woT_d = nc.dram_tensor("woT", [INNER, D], BF16, kind="ExternalInput")
    vnull_d = nc.dram_tensor("vnull", [DH], F32, kind="ExternalInput")
    mask_d = nc.dram_tensor("mask", [N], U8, kind="ExternalInput")
    gout_d = nc.dram_tensor("gout", [D], F32, kind="ExternalInput")
    out_d = nc.dram_tensor("out", [N, D], F32, kind="ExternalOutput")
    rdram = nc.dram_tensor("rscratch", [N], F32)   # r row round-trip
    nulldram = nc.dram_tensor("nullscratch", [H * N], BF16)  # unull reshape

    d_ = dict(xT_d=xT_d, x_d=x_d, wqT_d=wqT_d, wnT_d=wnT_d, wkvT_d=wkvT_d,
              woT_d=woT_d, vnull_d=vnull_d, mask_d=mask_d, gout_d=gout_d,
              out_d=out_d, rdram=rdram, nulldram=nulldram)
    with tile.TileContext(nc) as tc:
        _emit_tile(nc, tc, d_)
    return nc


def _emit_tile(nc, tc, d_):
    xT_d, x_d = d_["xT_d"], d_["x_d"]
    wqT_d, wnT_d, wkvT_d, woT_d = d_["wqT_d"], d_["wnT_d"], d_["wkvT_d"], d_["woT_d"]
    vnull_d, mask_d = d_["vnull_d"], d_["mask_d"]
    gout_d, out_d, rdram = d_["gout_d"], d_["out_d"], d_["rdram"]
    nulldram = d_["nulldram"]

    ctx = contextlib.ExitStack()
    with ctx:
        consts = ctx.enter_context(tc.tile_pool(name="consts", bufs=1))
        persist = ctx.enter_context(tc.tile_pool(name="persist", bufs=1))
        stage = ctx.enter_context(tc.tile_pool(name="stage", bufs=4))

        # ---------------- constants ----------------
        ident = consts.tile([128, 128], BF16)
        make_identity(nc, ident[:])
        identf = consts.tile([128, 128], F32)
        make_identity(nc, identf[:])
        # causal 0/1 band mask: keep u[j_rel, i_rel] iff i_rel >= j_rel
        mtri = consts.tile([128, 128], BF16)
        nc.gpsimd.memset(mtri[:], 1.0)
        nc.gpsimd.affine_select(out=mtri[:], in_=mtri[:], compare_op=ALU.is_ge,
                                fill=0.0, base=0, pattern=[[1, 128]],
                                channel_multiplier=-1)
        # broadcast pad mask: maskb[p, j] = mask_j (all partitions equal)
        maskb_u8 = consts.tile([128, N], U8)
        nc.sync.dma_start(out=maskb_u8[:],
                          in_=bass.AP(tensor=mask_d, offset=0,
                                      ap=[[0, 128], [1, N]]))
        maskb = consts.tile([128, N], BF16)
        nc.vector.tensor_copy(maskb[:], maskb_u8[:])
        # per-j-tile mask columns: maskc[p, t] = mask_{128t+p}
        maskc_u8 = consts.tile([128, NT], U8)
        nc.sync.dma_start(out=maskc_u8[:],
                          in_=bass.AP(tensor=mask_d, offset=0,
                                      ap=[[1, 128], [128, NT]]))
        maskc = consts.tile([128, NT], BF16)
        nc.vector.tensor_copy(maskc[:], maskc_u8[:])
        eps_t = consts.tile([128, 1], F32)
        nc.vector.memset(eps_t[:], EPS)
        ones_t = consts.tile([128, 2], BF16)
        nc.vector.memset(ones_t[:], 1.0)
        # warm the ACT tables (Sqrt/Exp) outside any dependency chain
        warm = consts.tile([128, 2], F32)
        nc.scalar.activation(out=warm[:, 0:1], in_=eps_t[:], func=AF.Sqrt)
        nc.scalar.activation(out=warm[:, 1:2], in_=eps_t[:], func=AF.Exp)

        # ------------- whole-kernel persistent tiles -------------
        kT2 = persist.tile([128, N], BF16, tag="kT2")       # k^T in both 64-halves
        v_nat = persist.tile([128, NT, DH + 1], BF16, tag="v_nat")  # col 64 = mask
        vnull16 = persist.tile([H, DH + 1], BF16, tag="vnull16")    # [v_null | 1] x16
        unull = persist.tile([H, N], BF16, tag="unull")     # null-token exp rows
        unull_r = persist.tile([1, H * N], BF16, tag="unull_r")  # partition-0 form
        outT = persist.tile([128, NT, NT, 128], BF16, tag="outT")  # [c-pair, it, i]
        rbroad = persist.tile([128, N], F32, tag="rbroad")  # rstd row broadcast
        qT = persist.tile([128, NT, N], BF16, tag="qT")     # q^T (pair slabs)
        xTs = persist.tile([128, NT, N], BF16, tag="xTs")   # x^T resident
        wqs = persist.tile([128, NT, INNER], BF16, tag="wqs")

        # ============ Phase A: projections + stats ============
        with tc.tile_pool(name="poolA", bufs=1) as poolA, \
             tc.tile_pool(name="psQ", bufs=3, space="PSUM") as psQ:
            wkvs = poolA.tile([128, NT, 2 * DH], BF16, tag="wkvs")
            wns = poolA.tile([128, NT, H], BF16, tag="wns")
            vts = poolA.tile([128, N], BF16, tag="vts")   # rows 64:128 = masked v^T
            rall = poolA.tile([128, NT], F32, tag="rall")

            with tc.tile_pool(name="psKV", bufs=1, space="PSUM") as psKV, \
                 tc.tile_pool(name="psVT", bufs=1, space="PSUM") as psVT:
                # --- loads (order matters: wkv, xT, x, wq, wn) ---
                nc.sync.dma_start(out=wkvs[:],
                                  in_=wkvT_d.ap().rearrange("(t p) c -> p t c", p=128))
                for t in range(NT):
                    nc.sync.dma_start(out=xTs[:, t, :],
                                      in_=xT_d[t * 128:(t + 1) * 128, :])

                # --- kv projection: kvT[c, j] accumulated over d-tiles ---
                pkv = psKV.tile([128, N], F32, tag="pkv")
                for t in range(NT):
                    for ch in range(2):
                        nc.tensor.matmul(pkv[:, ch * 512:(ch + 1) * 512],
                                         wkvs[:, t, :],
                                         xTs[:, t, ch * 512:(ch + 1) * 512],
                                         start=(t == 0), stop=(t == NT - 1))
                # evac: k^T into both halves (masked), v^T masked into rows 64:128
                nc.vector.scalar_tensor_tensor(out=kT2[0:64, :], in0=pkv[0:64, :],
                                               scalar=1.0, in1=maskb[0:64, :],
                                               op0=ALU.mult, op1=ALU.mult)
                nc.scalar.copy(kT2[64:128, :], kT2[0:64, :])
                nc.vector.scalar_tensor_tensor(out=vts[64:128, :], in0=pkv[64:128, :],
                                               scalar=1.0, in1=maskb[64:128, :],
                                               op0=ALU.mult, op1=ALU.mult)
                # v: transpose to natural [j, c] tiles (bf16 transposes, one bank)
                pvt = psVT.tile([128, NT, DH], BF16, tag="pvt")
                for t in range(NT):
                    nc.tensor.transpose(pvt[:, t, :],
                                        vts[64:128, t * 128:(t + 1) * 128],
                                        ident[64:128, 64:128])
                nc.vector.tensor_copy(v_nat[:, :, 0:DH], pvt[:])
                for t in range(NT):
                    nc.vector.tensor_copy(v_nat[:, t, DH:DH + 1], maskc[:, t:t + 1])
                # vnull16 = [v_null | 1] replicated on 16 partitions
                vn_s = stage.tile([H, DH], F32, tag="vn")
                nc.sync.dma_start(out=vn_s[:],
                                  in_=bass.AP(tensor=vnull_d, offset=0,
                                              ap=[[0, H], [1, DH]]))
                nc.vector.tensor_copy(vnull16[:, 0:DH], vn_s[:])
                nc.vector.tensor_copy(vnull16[:, DH:DH + 1], ones_t[0:H, 0:1])

                # --- LN1 stats from x tiles; r = rsqrt(var+eps) ---
                for it in range(NT):
                    x_s = stage.tile([128, D], BF16, tag="ldx")
                    nc.sync.dma_start(out=x_s[:], in_=x_d[it * 128:(it + 1) * 128, :])
                    st = stage.tile([128, 2, 6], F32, tag="bnst")
                    nc.vector.bn_stats(out=st[:, 0, :], in_=x_s[:, 0:512])
                    nc.vector.bn_stats(out=st[:, 1, :], in_=x_s[:, 512:1024])
                    mv = stage.tile([128, 2], F32, tag="bnmv")
                    nc.vector.bn_aggr(out=mv[:], in_=st[:])
                    nc.scalar.activation(out=rall[:, it:it + 1], in_=mv[:, 1:2],
                                         func=AF.Sqrt, bias=eps_t[:], scale=1.0)
                nc.vector.reciprocal(out=rall[:], in_=rall[:])
                # r -> row form via transpose + DRAM round-trip broadcast
                prT = psVT.tile([NT, 128], F32, tag="prT")
                nc.tensor.transpose(prT[:], rall[:], identf[:])
                rT_s = stage.tile([NT, 128], F32, tag="rTs")
                nc.vector.tensor_copy(rT_s[:], prT[:])
                nc.sync.dma_start(out=bass.AP(tensor=rdram, offset=0,
                                              ap=[[128, NT], [1, 128]]),
                                  in_=rT_s[:])
                nc.sync.dma_start(out=rbroad[:],
                                  in_=bass.AP(tensor=rdram, offset=0,
                                              ap=[[0, 128], [1, N]]))

            with tc.tile_pool(name="psNul", bufs=1, space="PSUM") as psNul:
                # --- q weights + projection: qT = r * (Wq'' @ x^T) ---
                for t in range(NT):
                    nc.sync.dma_start(out=wqs[:, t, :],
                                      in_=wqT_d[t * 128:(t + 1) * 128, :])
                nc.sync.dma_start(out=wns[:],
                                  in_=wnT_d.ap().rearrange("(t p) c -> p t c", p=128))
                for m in range(NT):
                    ms = slice(m * 128, (m + 1) * 128)
                    for ch in range(2):
                        pq = psQ.tile([128, 512], F32, tag="pq")
                        for t in range(NT):
                            nc.tensor.matmul(pq[:], wqs[:, t, ms],
                                             xTs[:, t, ch * 512:(ch + 1) * 512],
                                             start=(t == 0), stop=(t == NT - 1))
                        nc.vector.scalar_tensor_tensor(
                            out=qT[:, m, ch * 512:(ch + 1) * 512], in0=pq[:],
                            scalar=1.0, in1=rbroad[:, ch * 512:(ch + 1) * 512],
                            op0=ALU.mult, op1=ALU.mult)
                # --- null scores: unull[h, i] = exp(0.125 * r_i * (wn^T x_i)) ---
                pnull = psNul.tile([H, N], F32, tag="pnull")
                for t in range(NT):
                    for ch in range(2):
                        nc.tensor.matmul(pnull[:, ch * 512:(ch + 1) * 512],
                                         wns[:, t, :],
                                         xTs[:, t, ch * 512:(ch + 1) * 512],
                                         start=(t == 0), stop=(t == NT - 1))
                nl_s = stage.tile([H, N], F32, tag="nls")
                nc.vector.scalar_tensor_tensor(out=nl_s[:], in0=pnull[:], scalar=1.0,
                                               in1=rbroad[0:H, :],
                                               op0=ALU.mult, op1=ALU.mult)
                nc.scalar.activation(out=unull[:], in_=nl_s[:], func=AF.Exp,
                                     scale=0.125)
                # reshape to partition 0 (matmul lhsT base must be 0/32/64/96)
                nc.sync.dma_start(out=bass.AP(tensor=nulldram, offset=0,
                                              ap=[[N, H], [1, N]]),
                                  in_=unull[:])
                nc.sync.dma_start(out=unull_r[:],
                                  in_=bass.AP(tensor=nulldram, offset=0,
                                              ap=[[0, 1], [1, H * N]]))

        # ============ Phase B: attention ============
        with tc.tile_pool(name="psS", bufs=2, space="PSUM") as psS, \
             tc.tile_pool(name="psPV", bufs=2, space="PSUM") as psPV, \
             tc.tile_pool(name="psTR", bufs=2, space="PSUM") as psTR, \
             tc.tile_pool(name="upool", bufs=3) as upool, \
             tc.tile_pool(name="opool", bufs=2) as opool, \
             tc.tile_pool(name="rcpool", bufs=4) as rcpool:
            for m in range(NT):              # head pairs
                o_nat = opool.tile([128, NT, 128], BF16, tag="onat")
                for ph in range(2):          # parity within pair
                    h = 2 * m + ph
                    base = 64 * ph
                    u = upool.tile([128, U_COLS], BF16, tag="u")
                    for grp in EXP_GROUPS:
                        ps = psS.tile([128, N], F32, tag="scores")
                        goff = U_OFF[grp[0]]
                        for t in grp:
                            lo = 128 * t
                            co = U_OFF[t] - goff   # column offset inside group
                            w = N - lo
                            for c0 in range(0, w, 512):
                                cw = min(512, w - c0)
                                nc.tensor.matmul(
                                    ps[:, co + c0:co + c0 + cw],
                                    kT2[base:base + 64, lo:lo + 128],
                                    qT[base:base + 64, m, lo + c0:lo + c0 + cw],
                                    start=True, stop=True)
                        gw = sum(N - 128 * t for t in grp)
                        nc.scalar.activation(out=u[:, goff:goff + gw],
                                             in_=ps[:, 0:gw], func=AF.Exp,
                                             scale=0.125)
                        for t in grp:  # causal band of each tile in the group
                            nc.vector.tensor_mul(u[:, U_OFF[t]:U_OFF[t] + 128],
                                                 u[:, U_OFF[t]:U_OFF[t] + 128],
                                                 mtri[:])
                    # PV natural: out[i, c] += u_t[:, i-chunk]^T @ v_t
                    pvp = [psPV.tile([128, 4, DH + 1], F32, tag="pv", name="pv")
                           for _ in range(2)]
                    for it in range(NT):
                        pv = pvp[it // 4]
                        q_ = it % 4
                        for t in range(it + 1):
                            nc.tensor.matmul(
                                pv[:, q_, :],
                                u[:, U_OFF[t] + 128 * (it - t):U_OFF[t] + 128 * (it - t) + 128],
                                v_nat[:, t, :],
                                start=(t == 0), stop=False)
                        nc.tensor.matmul(
                            pv[:, q_, :],
                            unull_r[0:1, h * N + it * 128:h * N + it * 128 + 128],
                            vnull16[0:1, :],
                            start=False, stop=True)
                    # reciprocal of denominators (col 64), fused divide at evac
                    for half in range(2):
                        pv = pvp[half]
                        rc = rcpool.tile([128, 4, 1], F32, tag="rc")
                        nc.vector.reciprocal(out=rc[:], in_=pv[:, :, DH:DH + 1])
                        nc.vector.scalar_tensor_tensor(
                            out=o_nat[:, 4 * half:4 * half + 4, base:base + 64],
                            in0=pv[:, :, 0:DH], scalar=1.0,
                            in1=rc[:].broadcast_to([128, 4, DH]),
                            op0=ALU.mult, op1=ALU.mult)
                # transpose pair output back to [c, i] for the out-projection
                ptr = psTR.tile([128, NT, 128], BF16, tag="ptr")
                for it in range(NT):
                    nc.tensor.transpose(ptr[:, it, :], o_nat[:, it, :], ident[:])
                nc.vector.tensor_copy(outT[:, m, :, :], ptr[:])

        # ============ Phase C: out-projection + LN2 ============
        with tc.tile_pool(name="psC", bufs=4, space="PSUM") as psC, \
             tc.tile_pool(name="poolC", bufs=1) as poolC:
            wos = poolC.tile([128, NT, D], BF16, tag="wos")
            for t in range(NT):
                nc.sync.dma_start(out=wos[:, t, :], in_=woT_d[t * 128:(t + 1) * 128, :])
            gout_b = poolC.tile([128, D], F32, tag="gout_b")
            nc.sync.dma_start(out=gout_b[:],
                              in_=bass.AP(tensor=gout_d, offset=0,
                                          ap=[[0, 128], [1, D]]))
            for it in range(NT):
                i_s = slice(it * 128, (it + 1) * 128)
                st = stage.tile([128, 2, 6], F32, tag="bnst")
                pos = []
                for ch in range(2):
                    po = psC.tile([128, 512], F32, tag="po")
                    for ct in range(NT):
                        nc.tensor.matmul(po[:], outT[:, ct, it, :],
                                         wos[:, ct, ch * 512:(ch + 1) * 512],
                                         start=(ct == 0), stop=(ct == NT - 1))
                    nc.vector.bn_stats(out=st[:, ch, :], in_=po[:])
                    pos.append(po)
                mv = stage.tile([128, 2], F32, tag="bnmv")
                nc.vector.bn_aggr(out=mv[:], in_=st[:])
                rstd = stage.tile([128, 1], F32, tag="rstd")
                nc.scalar.activation(out=rstd[:], in_=mv[:, 1:2], func=AF.Sqrt,
                                     bias=eps_t[:], scale=1.0)
                nc.vector.reciprocal(out=rstd[:], in_=rstd[:])
                o_s = stage.tile([128, D], F32, tag="os")
                for ch in range(2):
                    cs = slice(ch * 512, (ch + 1) * 512)
                    nc.vector.tensor_scalar(out=o_s[:, cs], in0=pos[ch][:],
                                            scalar1=mv[:, 0:1], scalar2=rstd[:],
                                            op0=ALU.subtract, op1=ALU.mult)
                    nc.gpsimd.tensor_mul(o_s[:, cs], o_s[:, cs], gout_b[:, cs])
                    nc.sync.dma_start(out=out_d[it * 128:(it + 1) * 128, cs],
                                      in_=o_s[:, cs])


_CACHED = None


def _get_nc():
    global _CACHED
    if _CACHED is None:
        nc = bacc.Bacc("TRN2", target_bir_lowering=False, debug=False)
        _emit(nc)
        nc.compile()
        _CACHED = nc
    return _CACHED


def make_in_maps(x, mask, g_in, Wq, Wkv, null_kv, Wo, g_out):
    b = x.shape[0]
    BF = ml_dtypes.bfloat16
    g = g_in.astype(np.float64)
    W2 = Wq.astype(np.float64) * g[None, :]                  # [INNER, D]
    Wqq = W2 - W2.sum(axis=1, keepdims=True) / D             # fold mean removal
    wn = np.einsum('k,hkd->hd', null_kv[0].astype(np.float64),
                   Wqq.reshape(H, DH, D))                    # [H, D]
    xT = np.transpose(x, (0, 2, 1))
    mask_u8 = np.ascontiguousarray(mask).view(np.uint8) if mask.dtype == np.bool_ \
        else mask.astype(np.uint8)
    shared = {
        "wqT": np.ascontiguousarray(Wqq.T.astype(BF)),
        "wnT": np.ascontiguousarray(wn.T.astype(BF)),
        "wkvT": np.ascontiguousarray(Wkv.T.astype(BF)),
        "woT": np.ascontiguousarray(Wo.T.astype(BF)),
        "vnull": np.ascontiguousarray(null_kv[1].astype(np.float32)),
        "gout": np.ascontiguousarray(g_out.astype(np.float32)),
    }
    return [
        {"x": np.ascontiguousarray(x[c].astype(BF)),
         "xT": np.ascontiguousarray(xT[c].astype(BF)),
         "mask": mask_u8[c], **shared}
        for c in range(b)
    ]


def kernel(x, mask, g_in, Wq, Wkv, null_kv, Wo, g_out):
    x = np.asarray(x)
    mask = np.asarray(mask)
    g_in, g_out = np.asarray(g_in), np.asarray(g_out)
    Wq, Wkv, Wo = np.asarray(Wq), np.asarray(Wkv), np.asarray(Wo)
    null_kv = np.asarray(null_kv)
    b = x.shape[0]
    assert x.shape == (b, N, D) and b == 8
    in_maps = make_in_maps(x, mask, g_in, Wq, Wkv, null_kv, Wo, g_out)
    nc = _get_nc()
    res = run_bass_kernel_spmd(nc, in_maps, core_ids=list(range(b)))
    return np.stack([res.results[c]["out"] for c in range(b)], axis=0)


# revision 22
# speedup vs baseline: 1.3449x; 1.2174x over previous
"""Trainium2 Bass kernel for the masked MQA attention block (nn_Attention_4252017623134).

Sharding: pure data-parallel over batch. b=8 batch elements, 8 NeuronCores,
one batch element per core, weights replicated. No collectives.

Per-core math (n=1024, d=1024, h=16, dh=64, inner=1024):
  context = x                      (pre-norm residual branch feeds K/V)
  xn  = layernorm(x) * g_in
  q   = xn @ Wq.T   (per head, scaled by 1/8 = dh^-0.5, folded into exp scale)
  k,v = context @ Wkv.T (single shared KV head) + null_kv token
  att = softmax(mask(q k^T / 8))   (padding + causal(key j visible iff j <= i))
  out = layernorm(att @ v @ Wo.T) * g_out

Key design decisions:
  * All matmuls in bf16 (1 PE row/cycle at ANY width, vs f32r needing >=256).
  * LN1 folded into the q-projection: q_i = r_i * (Wq'' @ x_i) with
    Wq'' = Wq*diag(g) - outer(Wq@g, 1)/D precomputed on HOST (mean removal)
    and r_i = rsqrt(var_i+eps) applied as a per-column scale at PSUM evac.
    Removes all xn transposes and the LN->transpose->matmul serialization.
  * Null-token scores are 16 extra projection channels (wn = nk . Wq''_h,
    host-computed) -> no separate null-score machinery.
  * Padding mask applied by ZEROING masked k/v columns instead of an exp
    bias: masked j gives u=exp(0)=1 but contributes v_j=0 to the numerator
    and is excluded from the denominator via a mask column appended to V.
    Exp needs NO per-partition bias, so one activation call spans the score
    tiles of a whole multi-j-tile group.
  * Scores computed transposed (simT[j,i], exact visible windows); P@V runs
    NATURAL: lhsT = u[j, i-chunk], rhs = v_nat [j, 64ch + mask col]. Each
    accumulation step costs 65 cycles instead of an i-window: PV ~19us
    instead of ~37us. Softmax denominator lands in PSUM col 64; division is
    fused into the PV evac as a per-partition reciprocal multiply.
  * PV output [i, c] returns to [c, i] via XBAR DMA transposes (no PE/DVE).
  * Engine choreography: weight DMAs split across the SP and Activation
    HWDGE queues; per-pair emission interleaves q-proj -> scores -> PV so
    the Act engine (exp, the phase-B bottleneck) starts at ~14us and never
    starves; kT half-duplication via SBUF->SBUF DMA keeps Copy off the Act
    activation tables.
"""

import contextlib

import numpy as np
import ml_dtypes

import concourse.bass as bass
import concourse.bacc as bacc
import concourse.tile as tile
import concourse.mybir as mybir
from concourse.bass_utils import run_bass_kernel_spmd
from concourse.masks import make_identity

N = 1024          # sequence length per core
D = 1024          # model dim
H = 16            # query heads
DH = 64           # head dim
INNER = H * DH    # 1024
NT = N // 128     # 8 i-tiles / j-tiles / d-tiles
EPS = 1e-5

F32 = mybir.dt.float32
BF16 = mybir.dt.bfloat16
U8 = mybir.dt.uint8
AF = mybir.ActivationFunctionType
ALU = mybir.AluOpType

# exp groups per head: j-tiles packed into [128, 1024] fp32 (2-bank) PSUM
# tiles; every matmul segment stays inside a 512-col bank.
EXP_GROUPS = [(0,), (1, 7), (2, 6), (3, 5), (4,)]
U_OFF = {}
_off = 0
for _g in EXP_GROUPS:
    for _t in _g:
        U_OFF[_t] = _off
        _off += N - 128 * _t
U_COLS = _off  # 4608


def _bank_chunks(c0, c1):
    """Split [c0, c1) at 512-column (2KB fp32 PSUM bank) boundaries."""
    out = []
    while c0 < c1:
        nxt = min(c1, (c0 // 512 + 1) * 512)
        out.append((c0, nxt))
        c0 = nxt
    return out


def _emit(nc):
    # ---------------- DRAM I/O ----------------
    xT_d = nc.dram_tensor("xT", [D, N], BF16, kind="ExternalInput")
    x_d = nc.dram_tensor("x", [N, D], BF16, kind="ExternalInput")
    wqT_d = nc.dram_tensor("wqT", [D, INNER], BF16, kind="ExternalInput")
    wnT_d = nc.dram_tensor("wnT", [D, H], BF16, kind="ExternalInput")
    wkvT_d = nc.dram_tensor("wkvT", [D, 2 * DH], BF16, kind="ExternalInput")
    woT_d = nc.dram_tensor("woT", [INNER, D], BF16, kind="ExternalInput")
    vnull_d = nc.dram_tensor("vnull", [DH], F32, kind="ExternalInput")
    mask_d = nc.dram_tensor("mask", [N], U8, kind="ExternalInput")
    gout_d = nc.dram_tensor("gout", [D], F32, kind="ExternalInput")
    out_d = nc.dram_tensor("out", [N, D], F32, kind="ExternalOutput")
    rdram = nc.dram_tensor("rscratch", [N], F32)   # r row round-trip

    d_ = dict(xT_d=xT_d, x_d=x_d, wqT_d=wqT_d, wnT_d=wnT_d, wkvT_d=wkvT_d,
              woT_d=woT_d, vnull_d=vnull_d, mask_d=mask_d, gout_d=gout_d,
              out_d=out_d, rdram=rdram)
    with tile.TileContext(nc) as tc:
        _emit_tile(nc, tc, d_)
    return nc


def _emit_tile(nc, tc, d_):
    xT_d, x_d = d_["xT_d"], d_["x_d"]
    wqT_d, wnT_d, wkvT_d, woT_d = d_["wqT_d"], d_["wnT_d"], d_["wkvT_d"], d_["woT_d"]
    vnull_d, mask_d = d_["vnull_d"], d_["mask_d"]
    gout_d, out_d, rdram = d_["gout_d"], d_["out_d"], d_["rdram"]

    ctx = contextlib.ExitStack()
    with ctx:
        consts = ctx.enter_context(tc.tile_pool(name="consts", bufs=1))
        persist = ctx.enter_context(tc.tile_pool(name="persist", bufs=1))
        stage = ctx.enter_context(tc.tile_pool(name="stage", bufs=4))

        # ------------- persistent tiles -------------
        kT2 = persist.tile([128, N], BF16, tag="kT2")       # k^T in both halves
        v_nat = persist.tile([128, NT, DH + 1], BF16, tag="v_nat")  # col 64 = mask
        vnull16 = persist.tile([1, DH + 1], BF16, tag="vnull16")    # [v_null | 1]
        unull = persist.tile([H, N], BF16, tag="unull")     # null-token exp rows
        unull_r = persist.tile([1, H * N], BF16, tag="unull_r")  # partition-0 form
        outT = persist.tile([128, NT, NT, 128], BF16, tag="outT")  # [c-pair, it, i]
        rbroad = persist.tile([128, N], F32, tag="rbroad")  # rstd row broadcast
        qT = persist.tile([128, NT, N], BF16, tag="qT")     # q^T (pair slabs)
        xTs = persist.tile([128, NT, N], BF16, tag="xTs")   # x^T resident
        wqs = persist.tile([128, NT, INNER], BF16, tag="wqs")  # reused for Wo in C
        wkvs = persist.tile([128, NT, 2 * DH], BF16, tag="wkvs")
        wns = persist.tile([128, NT, H], BF16, tag="wns")
        vts = persist.tile([128, N], BF16, tag="vts")   # rows 64:128 = masked v^T
        rall = persist.tile([128, NT], F32, tag="rall")
        gout_b = persist.tile([128, D], F32, tag="gout_b")

        # ---- DMA issue: Act HWDGE queue (xT t0-3, wn, wq t4-7) ----
        for t in range(4):
            nc.scalar.dma_start(out=xTs[:, t, :], in_=xT_d[t * 128:(t + 1) * 128, :])
        nc.scalar.dma_start(out=wns[:],
                            in_=wnT_d.ap().rearrange("(t p) c -> p t c", p=128))
        for t in range(4, NT):
            nc.scalar.dma_start(out=wqs[:, t, :],
                                in_=wqT_d[t * 128:(t + 1) * 128, :])

        # ---- DMA issue: SP HWDGE queue (wkv, xT t4-7, x, wq t0-3, consts) ----
        nc.sync.dma_start(out=wkvs[:],
                          in_=wkvT_d.ap().rearrange("(t p) c -> p t c", p=128))
        for t in range(4, NT):
            nc.sync.dma_start(out=xTs[:, t, :], in_=xT_d[t * 128:(t + 1) * 128, :])
        x_tiles = []
        for it in range(NT):
            x_s = stage.tile([128, D], BF16, tag="ldx", name="x_s")
            nc.sync.dma_start(out=x_s[:], in_=x_d[it * 128:(it + 1) * 128, :])
            x_tiles.append(x_s)
        for t in range(4):
            nc.sync.dma_start(out=wqs[:, t, :],
                              in_=wqT_d[t * 128:(t + 1) * 128, :])
        vn_s = stage.tile([1, DH], F32, tag="vn")
        nc.sync.dma_start(out=vn_s[:],
                          in_=bass.AP(tensor=vnull_d, offset=0,
                                      ap=[[0, 1], [1, DH]]))
        maskb_u8 = consts.tile([128, N], U8)
        nc.sync.dma_start(out=maskb_u8[:],
                          in_=bass.AP(tensor=mask_d, offset=0,
                                      ap=[[0, 128], [1, N]]))
        maskc_u8 = consts.tile([128, NT], U8)
        nc.sync.dma_start(out=maskc_u8[:],
                          in_=bass.AP(tensor=mask_d, offset=0,
                                      ap=[[1, 128], [128, NT]]))
        nc.sync.dma_start(out=gout_b[:],
                          in_=bass.AP(tensor=gout_d, offset=0,
                                      ap=[[0, 128], [1, D]]))

        # ---------------- constants ----------------
        ident = consts.tile([128, 128], BF16)
        make_identity(nc, ident[:])
        identf = consts.tile([128, 128], F32)
        make_identity(nc, identf[:])
        # causal 0/1 band mask: keep u[j_rel, i_rel] iff i_rel >= j_rel
        mtri = consts.tile([128, 128], BF16)
        nc.gpsimd.memset(mtri[:], 1.0)
        nc.gpsimd.affine_select(out=mtri[:], in_=mtri[:], compare_op=ALU.is_ge,
                                fill=0.0, base=0, pattern=[[1, 128]],
                                channel_multiplier=-1)
        maskb = consts.tile([128, N], BF16)
        nc.vector.tensor_copy(maskb[:], maskb_u8[:])
        maskc = consts.tile([128, NT], BF16)
        nc.vector.tensor_copy(maskc[:], maskc_u8[:])
        eps_t = consts.tile([128, 1], F32)
        nc.vector.memset(eps_t[:], EPS)
        ones_t = consts.tile([128, 2], BF16)
        nc.vector.memset(ones_t[:], 1.0)
        # warm the ACT tables (Sqrt/Exp) outside any dependency chain
        warm = consts.tile([128, 2], F32)
        nc.scalar.activation(out=warm[:, 0:1], in_=eps_t[:], func=AF.Sqrt)
        nc.scalar.activation(out=warm[:, 1:2], in_=eps_t[:], func=AF.Exp)

        with tc.tile_pool(name="psQ", bufs=2, space="PSUM") as psQ:
            # ============ Phase A: kv/null projections + stats ============
            with tc.tile_pool(name="psKV", bufs=1, space="PSUM") as psKV, \
                 tc.tile_pool(name="psNul", bufs=1, space="PSUM") as psNul, \
                 tc.tile_pool(name="psVT", bufs=1, space="PSUM") as psVT:
                # --- LN1 stats first in the DVE stream; r = rsqrt(var+eps) ---
                for it in range(NT):
                    x_s = x_tiles[it]
                    st = stage.tile([128, 2, 6], F32, tag="bnst")
                    nc.vector.bn_stats(out=st[:, 0, :], in_=x_s[:, 0:512])
                    nc.vector.bn_stats(out=st[:, 1, :], in_=x_s[:, 512:1024])
                    mv = stage.tile([128, 2], F32, tag="bnmv")
                    nc.vector.bn_aggr(out=mv[:], in_=st[:])
                    nc.scalar.activation(out=rall[:, it:it + 1], in_=mv[:, 1:2],
                                         func=AF.Sqrt, bias=eps_t[:], scale=1.0)
                nc.vector.reciprocal(out=rall[:], in_=rall[:])

                # --- kv projection: kvT[c, j] accumulated over d-tiles ---
                pkv = psKV.tile([128, N], F32, tag="pkv")
                for t in range(NT):
                    for ch in range(2):
                        nc.tensor.matmul(pkv[:, ch * 512:(ch + 1) * 512],
                                         wkvs[:, t, :],
                                         xTs[:, t, ch * 512:(ch + 1) * 512],
                                         start=(t == 0), stop=(t == NT - 1))
                # --- null-score projection ---
                pnull = psNul.tile([H, N], F32, tag="pnull")
                for t in range(NT):
                    for ch in range(2):
                        nc.tensor.matmul(pnull[:, ch * 512:(ch + 1) * 512],
                                         wns[:, t, :],
                                         xTs[:, t, ch * 512:(ch + 1) * 512],
                                         start=(t == 0), stop=(t == NT - 1))

                # r -> row form: transpose + DRAM round-trip broadcast
                prT = psVT.tile([NT, 128], F32, tag="prT")
                nc.tensor.transpose(prT[:], rall[:], identf[:])
                rT_s = stage.tile([NT, 128], F32, tag="rTs")
                nc.vector.tensor_copy(rT_s[:], prT[:])
                nc.sync.dma_start(out=bass.AP(tensor=rdram, offset=0,
                                              ap=[[128, NT], [1, 128]]),
                                  in_=rT_s[:])
                nc.sync.dma_start(out=rbroad[:],
                                  in_=bass.AP(tensor=rdram, offset=0,
                                              ap=[[0, 128], [1, N]]))

                # evac: masked k^T (low half), masked v^T (rows 64:128)
                nc.vector.scalar_tensor_tensor(out=kT2[0:64, :], in0=pkv[0:64, :],
                                               scalar=1.0, in1=maskb[0:64, :],
                                               op0=ALU.mult, op1=ALU.mult)
                nc.sync.dma_start(out=kT2[64:128, :], in_=kT2[0:64, :])
                nc.vector.scalar_tensor_tensor(out=vts[64:128, :], in0=pkv[64:128, :],
                                               scalar=1.0, in1=maskb[64:128, :],
                                               op0=ALU.mult, op1=ALU.mult)
                # v -> natural [j, c] tiles
                pvt = psVT.tile([128, NT, DH], BF16, tag="pvt")
                for t in range(NT):
                    nc.tensor.transpose(pvt[:, t, :],
                                        vts[64:128, t * 128:(t + 1) * 128],
                                        ident[64:128, 64:128])
                nc.vector.tensor_copy(v_nat[:, :, 0:DH], pvt[:])
                for t in range(NT):
                    nc.vector.tensor_copy(v_nat[:, t, DH:DH + 1], maskc[:, t:t + 1])
                nc.vector.tensor_copy(vnull16[:, 0:DH], vn_s[:])
                nc.vector.tensor_copy(vnull16[:, DH:DH + 1], ones_t[0:1, 0:1])

                # --- null exp: unull[h,i] = exp(0.125 * r_i * pnull) ---
                nl_s = stage.tile([H, N], F32, tag="nls", bufs=1)
                nc.vector.scalar_tensor_tensor(out=nl_s[:], in0=pnull[:],
                                               scalar=1.0, in1=rbroad[0:H, :],
                                               op0=ALU.mult, op1=ALU.mult)
                nc.scalar.activation(out=unull[:], in_=nl_s[:], func=AF.Exp,
                                     scale=0.125)
                # reshape to partition 0 (matmul lhsT base must be 0/32/64/96)
                nc.sync.dma_start(out=unull_r[:], in_=unull[:])

            # ============ Phase B: q-proj + attention, per head pair ============
            with tc.tile_pool(name="psS", bufs=2, space="PSUM") as psS, \
                 tc.tile_pool(name="psPV", bufs=2, space="PSUM") as psPV, \
                 tc.tile_pool(name="upool", bufs=3) as upool, \
                 tc.tile_pool(name="opool", bufs=2) as opool, \
                 tc.tile_pool(name="rcpool", bufs=4) as rcpool:
                for m in range(NT):              # head pairs
                    # --- q-projection for this pair ---
                    ms = slice(m * 128, (m + 1) * 128)
                    for ch in range(2):
                        pq = psQ.tile([128, 512], F32, tag="pq")
                        for t in range(NT):
                            nc.tensor.matmul(pq[:], wqs[:, t, ms],
                                             xTs[:, t, ch * 512:(ch + 1) * 512],
                                             start=(t == 0), stop=(t == NT - 1))
                        nc.vector.scalar_tensor_tensor(
                            out=qT[:, m, ch * 512:(ch + 1) * 512], in0=pq[:],
                            scalar=1.0, in1=rbroad[:, ch * 512:(ch + 1) * 512],
                            op0=ALU.mult, op1=ALU.mult)
                    if m == NT - 1:
                        # wqs is dead after this pair's q-proj: refill with Wo
                        for t in range(NT):
                            nc.sync.dma_start(out=wqs[:, t, :],
                                              in_=woT_d[t * 128:(t + 1) * 128, :])
                    o_nat = opool.tile([128, NT, 128], BF16, tag="onat")
                    # --- scores + exp (both parities first: keeps Act fed) ---
                    us = []
                    for ph in range(2):
                        base = 64 * ph
                        u = upool.tile([128, U_COLS], BF16, tag="u", name="u")
                        us.append(u)
                        for grp in EXP_GROUPS:
                            ps = psS.tile([128, N], F32, tag="scores")
                            goff = U_OFF[grp[0]]
                            for t in grp:
                                lo = 128 * t
                                co = U_OFF[t] - goff
                                for c0, c1 in _bank_chunks(co, co + N - lo):
                                    nc.tensor.matmul(
                                        ps[:, c0:c1],
                                        kT2[base:base + 64, lo:lo + 128],
                                        qT[base:base + 64, m,
                                           lo + (c0 - co):lo + (c1 - co)],
                                        start=True, stop=True)
                            gw = sum(N - 128 * t for t in grp)
                            nc.scalar.activation(out=u[:, goff:goff + gw],
                                                 in_=ps[:, 0:gw], func=AF.Exp,
                                                 scale=0.125)
                            for t in grp:  # causal band of each tile in group
                                nc.vector.tensor_mul(u[:, U_OFF[t]:U_OFF[t] + 128],
                                                     u[:, U_OFF[t]:U_OFF[t] + 128],
                                                     mtri[:])
                    # --- PV natural + fused softmax division ---
                    for ph in range(2):
                        h = 2 * m + ph
                        base = 64 * ph
                        u = us[ph]
                        pvp = [psPV.tile([128, 4, DH + 1], F32, tag="pv", name="pv")
                               for _ in range(2)]
                        for it in range(NT):
                            pv = pvp[it // 4]
                            q_ = it % 4
                            for t in range(it + 1):
                                uo = U_OFF[t] + 128 * (it - t)
                                nc.tensor.matmul(pv[:, q_, :], u[:, uo:uo + 128],
                                                 v_nat[:, t, :],
                                                 start=(t == 0), stop=False)
                            nc.tensor.matmul(
                                pv[:, q_, :],
                                unull_r[0:1, h * N + it * 128:h * N + it * 128 + 128],
                                vnull16[0:1, :],
                                start=False, stop=True)
                        for half in range(2):
                            pv = pvp[half]
                            rc = rcpool.tile([128, 4, 1], F32, tag="rc")
                            nc.vector.reciprocal(out=rc[:], in_=pv[:, :, DH:DH + 1])
                            nc.vector.scalar_tensor_tensor(
                                out=o_nat[:, 4 * half:4 * half + 4, base:base + 64],
                                in0=pv[:, :, 0:DH], scalar=1.0,
                                in1=rc[:].broadcast_to([128, 4, DH]),
                                op0=ALU.mult, op1=ALU.mult)
                    # --- pair output back to [c, i] via XBAR DMA transpose ---
                    for it in range(NT):
                        nc.sync.dma_start_transpose(out=outT[:, m, it, :],
                                                    in_=o_nat[:, it, :])

        # ============ Phase C: out-projection + LN2 ============
        with tc.tile_pool(name="psC", bufs=4, space="PSUM") as psC:
            for it in range(NT):
                st = stage.tile([128, 2, 6], F32, tag="bnst")
                pos = []
                for ch in range(2):
                    po = psC.tile([128, 512], F32, tag="po")
                    for ct in range(NT):
                        nc.tensor.matmul(po[:], outT[:, ct, it, :],
                                         wqs[:, ct, ch * 512:(ch + 1) * 512],
                                         start=(ct == 0), stop=(ct == NT - 1))
                    nc.vector.bn_stats(out=st[:, ch, :], in_=po[:])
                    pos.append(po)
                mv = stage.tile([128, 2], F32, tag="bnmv")
                nc.vector.bn_aggr(out=mv[:], in_=st[:])
                rstd = stage.tile([128, 1], F32, tag="rstd")
                nc.scalar.activation(out=rstd[:], in_=mv[:, 1:2], func=AF.Sqrt,
                                     bias=eps_t[:], scale=1.0)
                nc.vector.reciprocal(out=rstd[:], in_=rstd[:])
                o_s = stage.tile([128, D], F32, tag="os", bufs=2)
                for ch in range(2):
                    cs = slice(ch * 512, (ch + 1) * 512)
                    nc.vector.tensor_scalar(out=o_s[:, cs], in0=pos[ch][:],
                                            scalar1=mv[:, 0:1], scalar2=rstd[:],
                                            op0=ALU.subtract, op1=ALU.mult)
                    nc.gpsimd.tensor_mul(o_s[:, cs], o_s[:, cs], gout_b[:, cs])
                    nc.sync.dma_start(out=out_d[it * 128:(it + 1) * 128, cs],
                                      in_=o_s[:, cs])


_CACHED = None


def _get_nc():
    global _CACHED
    if _CACHED is None:
        nc = bacc.Bacc("TRN2", target_bir_lowering=False, debug=False)
        _emit(nc)
        nc.compile()
        _CACHED = nc
    return _CACHED


def make_in_maps(x, mask, g_in, Wq, Wkv, null_kv, Wo, g_out):
    b = x.shape[0]
    BF = ml_dtypes.bfloat16
    g = g_in.astype(np.float64)
    W2 = Wq.astype(np.float64) * g[None, :]                  # [INNER, D]
    Wqq = W2 - W2.sum(axis=1, keepdims=True) / D             # fold mean removal
    wn = np.einsum('k,hkd->hd', null_kv[0].astype(np.float64),
                   Wqq.reshape(H, DH, D))                    # [H, D]
    xT = np.transpose(x, (0, 2, 1))
    mask_u8 = np.ascontiguousarray(mask).view(np.uint8) if mask.dtype == np.bool_ \
        else mask.astype(np.uint8)
    shared = {
        "wqT": np.ascontiguousarray(Wqq.T.astype(BF)),
        "wnT": np.ascontiguousarray(wn.T.astype(BF)),
        "wkvT": np.ascontiguousarray(Wkv.T.astype(BF)),
        "woT": np.ascontiguousarray(Wo.T.astype(BF)),
        "vnull": np.ascontiguousarray(null_kv[1].astype(np.float32)),
        "gout": np.ascontiguousarray(g_out.astype(np.float32)),
    }
    return [
        {"x": np.ascontiguousarray(x[c].astype(BF)),
         "xT": np.ascontiguousarray(xT[c].astype(BF)),
         "mask": mask_u8[c], **shared}
        for c in range(b)
    ]


def kernel(x, mask, g_in, Wq, Wkv, null_kv, Wo, g_out):
    x = np.asarray(x)
    mask = np.asarray(mask)
    g_in, g_out = np.asarray(g_in), np.asarray(g_out)
    Wq, Wkv, Wo = np.asarray(Wq), np.asarray(Wkv), np.asarray(Wo)
    null_kv = np.asarray(null_kv)
    b = x.shape[0]
    assert x.shape == (b, N, D) and b == 8
    in_maps = make_in_maps(x, mask, g_in, Wq, Wkv, null_kv, Wo, g_out)
    nc = _get_nc()
    res = run_bass_kernel_spmd(nc, in_maps, core_ids=list(range(b)))
    return np.stack([res.results[c]["out"] for c in range(b)], axis=0)
